# revision 6
# baseline (speedup 1.0000x reference)
"""Trainium2 Bass kernel for nn_Attention_90787018703157 (sparse_attention).

Reference computation (per batch element b):
    q = s @ Wq.T                      # [N, 32]
    k = s @ Wk.T                      # [N, 32]
    logits = q @ k.T                  # [N, N]
    w = logits**2 * G
    out = w / (w.sum(-1, keepdims=True) + 1e-6)

Sharding: data-parallel over the batch dim — B=8 batch elements, one per
NeuronCore.  Wq/Wk are replicated.

Precision strategy (correctness gate is rel_l2 < 2e-2, so 16-bit HBM
staging is safe by a wide margin):
  - G is cast to bf16 on the HOST and staged in HBM as bf16 (8 MiB/core
    instead of 16 MiB).  bf16 keeps fp32-range exponents, so no
    denormal-flush risk (unlike fp16 for outputs ~1e-5).
  - The output is computed in fp32 on-chip, written to HBM as bf16
    (8 MiB/core), and widened back to fp32 on the host.
  - HBM traffic per core per pass: 16 MiB -> ~47 us floor at 358 GB/s
    (vs ~94 us for the fp32 baseline).
  - Main q@kT matmuls run as float32r (bitcast view of the fp32 data):
    1 PE cycle/row at FD=512 instead of 4 for plain fp32.

Per-core plan:
  preamble (once, pipelined per 512-col m-block):
    sT  = s.T               via 16 PE transposes ([128,10] -> [10,128])
    qT  = Wq @ sT           via PE (K=10), kT likewise  -> SBUF [32, N]
  main loop over 8 pairs of row-blocks (2 x 128 rows, 1 MiB G per DMA):
    logits_ps[128, 2048] = qT_blk.T @ kT   (4 f32r matmuls, K=32)
    sq   = Square(logits_ps)               (ScalarE, PSUM->SBUF, bf16 out)
    o,rs = sq * G_blk, rowsum fused        (VectorE scalar_tensor_tensor,
                                            bf16 in/out, fp32 accum)
    rc   = 1/(rs + 1e-6)                   (VectorE)
    o   *= rc  in place                    (VectorE tensor_scalar, bf16 4x)
"""

from contextlib import ExitStack

import numpy as np

import concourse.bass as bass
import concourse.bacc as bacc
import concourse.tile as tile
from concourse import mybir
from concourse.bass_utils import run_bass_kernel_spmd
from concourse.masks import make_identity

B = 8
N = 2048
IN_DIM = 10
QK = 32
P = 128
NT = N // P      # 16 row blocks per core
MB = 512         # max moving free dim for fp32 matmul
NMB = N // MB    # 4
F32 = mybir.dt.float32
F32R = mybir.dt.float32r
BF16 = mybir.dt.bfloat16
EPS = 1e-6


def _np_bf16():
    import ml_dtypes

    return np.dtype(ml_dtypes.bfloat16)


def _build_nc(loop_reps: int = 1, hw_loop: bool = False) -> bass.Bass:
    # Bacc (not plain Bass): its finalize() runs move_matmul_waits_to_ldweights
    # + generate_event_semaphores, which split multi-wait instructions to
    # satisfy the TRN2 one-wait-per-instruction constraint.
    nc = bacc.Bacc()

    s_d = nc.dram_tensor("s", [N, IN_DIM], F32, kind="ExternalInput")
    G_d = nc.dram_tensor("G", [N, N], BF16, kind="ExternalInput")
    wq_d = nc.dram_tensor("Wq", [QK, IN_DIM], F32, kind="ExternalInput")
    wk_d = nc.dram_tensor("Wk", [QK, IN_DIM], F32, kind="ExternalInput")
    out_d = nc.dram_tensor("out", [N, N], BF16, kind="ExternalOutput")

    with tile.TileContext(nc) as tc, ExitStack() as ctx:
        consts = ctx.enter_context(tc.tile_pool(name="consts", bufs=1))

        ident = consts.tile([P, P], F32)
        make_identity(nc, ident)

        wqT = consts.tile([IN_DIM, QK], F32)
        nc.sync.dma_start(out=wqT, in_=wq_d.rearrange("q i -> i q"))
        wkT = consts.tile([IN_DIM, QK], F32)
        nc.sync.dma_start(out=wkT, in_=wk_d.rearrange("q i -> i q"))

        # s loaded so that row-block t sits at free-dim slot t: [128, 16, 10];
        # split per m-block so the transpose chain starts after 1/4 arrives.
        s_sb = consts.tile([P, NT, IN_DIM], F32)
        s_v = s_d.rearrange("(t p) i -> p t i", p=P)
        for m in range(NMB):
            nc.sync.dma_start(
                out=s_sb[:, 4 * m : 4 * m + 4, :], in_=s_v[:, 4 * m : 4 * m + 4, :]
            )

        sT = consts.tile([IN_DIM, N], F32)
        # f32r tiles: the PSUM->SBUF copies round to fp32r, which the BIR
        # verifier requires for operands of fp32r matmuls (1 PE cycle/row
        # instead of 4 for plain fp32).
        qT = consts.tile([QK, N], F32R)
        kT = consts.tile([QK, N], F32R)

        # Per 512-col m-block: 4 PE transposes -> sT slice -> q/k projection
        # matmuls -> SBUF, pipelined so the main loop can start after m=0.
        with tc.tile_pool(name="pre_ps", bufs=2, space="PSUM") as pre_ps:
            for m in range(NMB):
                sl = slice(m * MB, (m + 1) * MB)
                tr_ps = pre_ps.tile([IN_DIM, MB], F32, tag="tr", name="tr_ps")
                for j in range(4):
                    t = 4 * m + j
                    nc.tensor.transpose(
                        tr_ps[:, j * P : (j + 1) * P], s_sb[:, t, :], ident
                    )
                nc.scalar.copy(sT[:, sl], tr_ps)
                q_ps = pre_ps.tile([QK, MB], F32, tag="qps", name="q_ps")
                nc.tensor.matmul(q_ps, wqT, sT[:, sl])
                nc.vector.tensor_copy(qT[:, sl], q_ps)
                k_ps = pre_ps.tile([QK, MB], F32, tag="kps", name="k_ps")
                nc.tensor.matmul(k_ps, wkT, sT[:, sl])
                nc.scalar.copy(kT[:, sl], k_ps)

        # 2 row-blocks (1 MiB bf16) per DMA.  Loads alternate between the
        # two physical HWDGE rings (SP and ACT); stores go via the SWDGE
        # (gpsimd) path, keeping three DMA issue paths in parallel.
        BPD = 2  # blocks per DMA
        G_v = G_d.rearrange("(u b p) m -> u p b m", p=P, b=BPD)
        o_v = out_d.rearrange("(u b p) m -> u p b m", p=P, b=BPD)

        g_pool = ctx.enter_context(tc.tile_pool(name="g", bufs=4))
        sq_pool = ctx.enter_context(tc.tile_pool(name="sq", bufs=2))
        o_pool = ctx.enter_context(tc.tile_pool(name="o", bufs=3))
        small = ctx.enter_context(tc.tile_pool(name="small", bufs=4))
        ps_pool = ctx.enter_context(tc.tile_pool(name="ps", bufs=2, space="PSUM"))

        def one_pass():
            for u in range(NT // BPD):
                g2 = g_pool.tile([P, BPD, N], BF16, name="g2")
                (nc.sync if u % 2 == 0 else nc.scalar).dma_start(
                    out=g2, in_=G_v[u]
                )
                o2 = o_pool.tile([P, BPD, N], BF16, name="o2")

                for b in range(BPD):
                    t = BPD * u + b
                    lg = ps_pool.tile([P, N], F32, name="lg")
                    for m in range(NMB):
                        sl = slice(m * MB, (m + 1) * MB)
                        nc.tensor.matmul(
                            lg[:, sl], qT[:, t * P : (t + 1) * P], kT[:, sl]
                        )

                    sq_t = sq_pool.tile([P, N], BF16, name="sq_t")
                    nc.scalar.activation(
                        sq_t, lg, mybir.ActivationFunctionType.Square
                    )

                    # w = sq * G written straight into the output tile,
                    # rs = rowsum(w) fused in (fp32 accumulator)
                    rs = small.tile([P, 1], F32, tag="rs", name="rs")
                    nc.vector.scalar_tensor_tensor(
                        out=o2[:, b, :],
                        in0=sq_t,
                        scalar=1.0,
                        in1=g2[:, b, :],
                        op0=mybir.AluOpType.mult,
                        op1=mybir.AluOpType.mult,
                        accum_out=rs,
                    )
                    rse = small.tile([P, 1], F32, tag="rse", name="rse")
                    nc.vector.tensor_scalar_add(rse, rs, EPS)
                    rc = small.tile([P, 1], F32, tag="rc", name="rc")
                    nc.vector.reciprocal(rc, rse)

                    # in-place per-row scale on DVE (bf16 in/out -> 4x mode)
                    nc.vector.tensor_scalar_mul(o2[:, b, :], o2[:, b, :], rc)

                nc.gpsimd.dma_start(out=o_v[u], in_=o2)

        if hw_loop and loop_reps > 1:
            with tc.For_i(0, loop_reps, 1):
                one_pass()
        else:
            for _ in range(loop_reps):
                one_pass()

    nc.finalize()
    return nc


_NC_CACHE = {}


def _get_nc(loop_reps: int = 1, hw_loop: bool = False) -> bass.Bass:
    key = (loop_reps, hw_loop)
    if key not in _NC_CACHE:
        _NC_CACHE[key] = _build_nc(loop_reps, hw_loop)
    return _NC_CACHE[key]


def _in_maps(inputs):
    s = np.ascontiguousarray(np.asarray(inputs["s"], dtype=np.float32))
    G = np.asarray(inputs["G"])
    Wq = np.ascontiguousarray(np.asarray(inputs["Wq"], dtype=np.float32))
    Wk = np.ascontiguousarray(np.asarray(inputs["Wk"], dtype=np.float32))
    assert s.shape == (B, N, IN_DIM), s.shape
    assert G.shape == (B, N, N), G.shape
    G16 = np.ascontiguousarray(G.astype(_np_bf16()))
    return [{"s": s[b], "G": G16[b], "Wq": Wq, "Wk": Wk} for b in range(B)]


def _run(inputs, trace: bool = False):
    nc = _get_nc()
    in_maps = _in_maps(inputs)
    res = run_bass_kernel_spmd(nc, in_maps, core_ids=list(range(B)), trace=trace)
    out = np.stack(
        [res.results[b]["out"].astype(np.float32) for b in range(B)], axis=0
    )
    return out, res


def kernel(s, G, Wq, Wk):
    out, _ = _run({"s": s, "G": G, "Wq": Wq, "Wk": Wk})
    return out


# revision 22
# speedup vs baseline: 3.2124x; 3.2124x over previous
"""Trainium2 Bass kernel for nn_Attention_90787018703157 (sparse_attention).

Reference computation (per batch element b):
    q = s @ Wq.T                      # [N, 32]
    k = s @ Wk.T                      # [N, 32]
    logits = q @ k.T                  # [N, N]
    w = logits**2 * G
    out = w / (w.sum(-1, keepdims=True) + 1e-6)

Sharding: data-parallel over the batch dim — B=8 batch elements, one per
NeuronCore.  Wq/Wk are replicated.

Precision strategy (correctness gate is rel_l2 < 2e-2; measured 3.5e-3):
  - G is quantized to uint8 on the HOST (round(G*255); the row
    normalization cancels the constant 255, and the quantization error
    enters weighted by w itself, so the small-G relative blowup cancels:
    ~0.2% rel_l2).  G HBM traffic: 4 MiB/core instead of 16.
  - The output is computed in fp32 on-chip, written to HBM as bf16
    (8 MiB/core instead of 16), and widened back to fp32 on the host.
  - HBM traffic per core per pass: 12 MiB -> ~35 us floor at 358 GB/s
    (vs ~94 us for the fp32 baseline).
  - Main q@kT matmuls run as float32r tiles: 1 PE cycle/row at FD=512
    instead of 4 for plain fp32 (the PSUM->SBUF copies do the f32r
    rounding the BIR verifier requires of f32r matmul operands).

Per-core plan:
  preamble (once, pipelined per 512-col m-block):
    sT  = s.T               via 16 PE transposes ([128,10] -> [10,128])
    qT  = Wq @ sT           via PE (K=10), kT likewise  -> SBUF [32, N]
  main loop over 8 pairs of row-blocks (2 x 128 rows, 0.5 MiB G per DMA):
    logits_ps[128, 2048] = qT_blk.T @ kT   (4 f32r matmuls, K=32)
    sq   = Square(logits_ps)               (ScalarE, PSUM->SBUF, bf16 out)
    o,rs = sq * G_blk, rowsum fused        (VectorE scalar_tensor_tensor,
                                            u8 G read directly, fp32 accum)
    rc   = 1/rs                            (VectorE; eps dropped, rs ~ 1e7)
    o   *= rc  in place                    (VectorE tensor_scalar, bf16 4x)

Measured steady state (For_i delta method): ~58.5 us/pass/core, vs the
fp32 baseline's ~114.5 us and this kernel's ~35 us DMA floor (DVE is the
binding engine at ~46 us busy; remaining gap is DMA/sync pipelining).
A/B results that did NOT survive: all-loads-on-one-HWDGE-ring (+4 us),
2 MiB load DMAs (+9 us), moving scale passes to ScalarE (+17 us here —
the rc-dependent scale head-of-line blocks Squares in the ACT queue).
"""

from contextlib import ExitStack

import numpy as np

import concourse.bass as bass
import concourse.bacc as bacc
import concourse.tile as tile
from concourse import mybir
from concourse.bass_utils import run_bass_kernel_spmd
from concourse.masks import make_identity

B = 8
N = 2048
IN_DIM = 10
QK = 32
P = 128
NT = N // P      # 16 row blocks per core
MB = 512         # max moving free dim for fp32 matmul
NMB = N // MB    # 4
F32 = mybir.dt.float32
F32R = mybir.dt.float32r
BF16 = mybir.dt.bfloat16
U8 = mybir.dt.uint8
EPS = 1e-6


def _build_nc(
    loop_reps: int = 1, hw_loop: bool = False, keep_tc: dict | None = None
) -> bass.Bass:
    # Bacc (not plain Bass): its finalize() runs move_matmul_waits_to_ldweights
    # + generate_event_semaphores, which split multi-wait instructions to
    # satisfy the TRN2 one-wait-per-instruction constraint.
    nc = bacc.Bacc()

    s_d = nc.dram_tensor("s", [N, IN_DIM], F32, kind="ExternalInput")
    G_d = nc.dram_tensor("G", [N, N], U8, kind="ExternalInput")
    wq_d = nc.dram_tensor("Wq", [QK, IN_DIM], F32, kind="ExternalInput")
    wk_d = nc.dram_tensor("Wk", [QK, IN_DIM], F32, kind="ExternalInput")
    out_d = nc.dram_tensor("out", [N, N], BF16, kind="ExternalOutput")

    with tile.TileContext(nc) as tc, ExitStack() as ctx:
        if keep_tc is not None:
            keep_tc["tc"] = tc
        consts = ctx.enter_context(tc.tile_pool(name="consts", bufs=1))

        ident = consts.tile([P, P], F32)
        make_identity(nc, ident)

        wqT = consts.tile([IN_DIM, QK], F32)
        nc.sync.dma_start(out=wqT, in_=wq_d.rearrange("q i -> i q"))
        wkT = consts.tile([IN_DIM, QK], F32)
        nc.sync.dma_start(out=wkT, in_=wk_d.rearrange("q i -> i q"))

        # s loaded so that row-block t sits at free-dim slot t: [128, 16, 10];
        # split per m-block so the transpose chain starts after 1/4 arrives.
        s_sb = consts.tile([P, NT, IN_DIM], F32)
        s_v = s_d.rearrange("(t p) i -> p t i", p=P)
        for m in range(NMB):
            nc.sync.dma_start(
                out=s_sb[:, 4 * m : 4 * m + 4, :], in_=s_v[:, 4 * m : 4 * m + 4, :]
            )

        sT = consts.tile([IN_DIM, N], F32)
        # f32r tiles: the PSUM->SBUF copies round to fp32r, which the BIR
        # verifier requires for operands of fp32r matmuls (1 PE cycle/row
        # instead of 4 for plain fp32).
        qT = consts.tile([QK, N], F32R)
        kT = consts.tile([QK, N], F32R)

        # Per 512-col m-block: 4 PE transposes -> sT slice -> q/k projection
        # matmuls -> SBUF, pipelined so the main loop can start after m=0.
        with tc.tile_pool(name="pre_ps", bufs=2, space="PSUM") as pre_ps:
            for m in range(NMB):
                sl = slice(m * MB, (m + 1) * MB)
                tr_ps = pre_ps.tile([IN_DIM, MB], F32, tag="tr", name="tr_ps")
                for j in range(4):
                    t = 4 * m + j
                    nc.tensor.transpose(
                        tr_ps[:, j * P : (j + 1) * P], s_sb[:, t, :], ident
                    )
                nc.scalar.copy(sT[:, sl], tr_ps)
                q_ps = pre_ps.tile([QK, MB], F32, tag="qps", name="q_ps")
                nc.tensor.matmul(q_ps, wqT, sT[:, sl])
                nc.vector.tensor_copy(qT[:, sl], q_ps)
                k_ps = pre_ps.tile([QK, MB], F32, tag="kps", name="k_ps")
                nc.tensor.matmul(k_ps, wkT, sT[:, sl])
                nc.scalar.copy(kT[:, sl], k_ps)

        # 2 row-blocks (1 MiB bf16) per load DMA, alternating the two
        # physical HWDGE rings (SP and ACT; a single ring serializes the
        # loads and measures ~4 us/pass slower).  Stores: 2 row-blocks
        # (1 MiB bf16) per SWDGE DMA on the gpsimd path.
        BPD = 2  # row-blocks per DMA
        G_v = G_d.rearrange("(u b p) m -> u p b m", p=P, b=BPD)
        o_v = out_d.rearrange("(u b p) m -> u p b m", p=P, b=BPD)

        g_pool = ctx.enter_context(tc.tile_pool(name="g", bufs=4))
        sq_pool = ctx.enter_context(tc.tile_pool(name="sq", bufs=2))
        o_pool = ctx.enter_context(tc.tile_pool(name="o", bufs=3))
        small = ctx.enter_context(tc.tile_pool(name="small", bufs=4))
        ps_pool = ctx.enter_context(tc.tile_pool(name="ps", bufs=2, space="PSUM"))

        def one_pass():
            for u in range(NT // BPD):
                g2 = g_pool.tile([P, BPD, N], U8, name="g2")
                (nc.sync if u % 2 == 0 else nc.scalar).dma_start(
                    out=g2, in_=G_v[u]
                )
                o2 = o_pool.tile([P, BPD, N], BF16, name="o2")

                for b in range(BPD):
                    t = BPD * u + b
                    lg = ps_pool.tile([P, N], F32, name="lg")
                    for m in range(NMB):
                        sl = slice(m * MB, (m + 1) * MB)
                        nc.tensor.matmul(
                            lg[:, sl], qT[:, t * P : (t + 1) * P], kT[:, sl]
                        )

                    sq_t = sq_pool.tile([P, N], BF16, name="sq_t")
                    nc.scalar.activation(
                        sq_t, lg, mybir.ActivationFunctionType.Square
                    )

                    # w = sq * G into the output tile, rs = rowsum(w)
                    # fused (fp32 accum).  eps is dropped: rs is ~3e4
                    # here, far above any rounding.
                    rs = small.tile([P, 1], F32, tag="rs", name="rs")
                    nc.vector.scalar_tensor_tensor(
                        out=o2[:, b, :],
                        in0=sq_t,
                        scalar=1.0,
                        in1=g2[:, b, :],
                        op0=mybir.AluOpType.mult,
                        op1=mybir.AluOpType.mult,
                        accum_out=rs,
                    )
                    rc = small.tile([P, 1], F32, tag="rc", name="rc")
                    nc.vector.reciprocal(rc, rs)

                    # per-row scale on DVE (bf16 in/out -> 4x mode).
                    # Splitting some of these onto ScalarE measured 17 us
                    # SLOWER: the scale waits on rc and head-of-line
                    # blocks the next block's Square in the ACT queue.
                    nc.vector.tensor_scalar_mul(o2[:, b, :], o2[:, b, :], rc)

                nc.gpsimd.dma_start(out=o_v[u], in_=o2)

        if hw_loop and loop_reps > 1:
            with tc.For_i(0, loop_reps, 1):
                one_pass()
        else:
            for _ in range(loop_reps):
                one_pass()

    nc.finalize()
    return nc


_NC_CACHE = {}


def _get_nc(loop_reps: int = 1, hw_loop: bool = False) -> bass.Bass:
    key = (loop_reps, hw_loop)
    if key not in _NC_CACHE:
        _NC_CACHE[key] = _build_nc(loop_reps, hw_loop)
    return _NC_CACHE[key]


def _in_maps(inputs):
    s = np.ascontiguousarray(np.asarray(inputs["s"], dtype=np.float32))
    G = np.asarray(inputs["G"])
    Wq = np.ascontiguousarray(np.asarray(inputs["Wq"], dtype=np.float32))
    Wk = np.ascontiguousarray(np.asarray(inputs["Wk"], dtype=np.float32))
    assert s.shape == (B, N, IN_DIM), s.shape
    assert G.shape == (B, N, N), G.shape
    # quantize G to u8: the kernel computes w = sq * (255*G) and the
    # row normalization cancels the constant 255
    Gq = np.ascontiguousarray(
        np.rint(np.asarray(G, dtype=np.float32) * 255.0).astype(np.uint8)
    )
    return [{"s": s[b], "G": Gq[b], "Wq": Wq, "Wk": Wk} for b in range(B)]


def _run(inputs, trace: bool = False):
    nc = _get_nc()
    in_maps = _in_maps(inputs)
    res = run_bass_kernel_spmd(nc, in_maps, core_ids=list(range(B)), trace=trace)
    out = np.stack(
        [res.results[b]["out"].astype(np.float32) for b in range(B)], axis=0
    )
    return out, res


def kernel(s, G, Wq, Wk):
    out, _ = _run({"s": s, "G": G, "Wq": Wq, "Wk": Wk})
    return out
